# revision 20
# baseline (speedup 1.0000x reference)
"""Trainium2 Bass kernel for nn_CheckEmotion: embedding gather -> LSTM(128) -> linear(28).

Strategy (data-parallel over 8 NeuronCores, 64 batch rows each):
  - Embedding gather on-device via indirect DMA (8192 tokens/block, 8 blocks),
    DMA-xbar transposes to xeT [E+1, tokens] (token-major). The padded
    embedding table carries a constant-1.0 column, and w_ih gets an extra
    bias row, so the LSTM bias rides the bulk gx matmul for free.
  - Input projections gx computed in bulk per 4-step window directly into PSUM
    (one bf16 matmul per gate, K=65, N=256), interleaved into the previous
    window's step tails.
  - PSUM layout is step-major so per-step activation reads are contiguous:
    ps_if packs (i,f) pairs per step in one bank; ps_go packs g and o halves
    in a second bank. Cell state c lives in PSUM (faster Activation access).
  - Recurrence: per step 4 bf16 gate matmuls (K=128, N=64) accumulate onto gx.
    sigmoid(i,f) fires right after the f matmul; sigmoid(o) off-chain.
    A tiny "warm" matmul on tanh(c) keeps the PE clock from dropping between
    steps. Cell/hidden updates on DVE; h carried in bf16, c in fp32.
  - Final linear computed transposed ([28, 64] = w_lin @ h.T + b); host
    transposes back.
"""
import sys
import numpy as np
from contextlib import ExitStack

sys.path.insert(0, '/opt/trn_rl_repo')

import concourse.bass as bass
import concourse.tile as tile
from concourse import bacc, mybir
from concourse.bass_utils import run_bass_kernel_spmd

V, E, H, NCLS = 50257, 64, 128, 28
B, T = 512, 1024
NCORES = 8
BC = B // NCORES            # 64 batch rows per core
TOK = BC * T                # 65536 tokens per core
BLK = 8192                  # tokens per gather block
NBLK = TOK // BLK           # 8
WSTEPS = 4                  # steps per psum window
NWIN = T // WSTEPS          # 256
WPB = BLK // (WSTEPS * BC)  # 32 windows per block
TPB = BLK // 128            # gather tiles per block (64)

F32 = mybir.dt.float32
BF16 = mybir.dt.bfloat16
I32 = mybir.dt.int32
EP = 128                    # padded embedding row (bf16) for 16-bit DMA transpose
KE = E + 1                  # contraction for gx (embedding + ones row for bias)

SIG = mybir.ActivationFunctionType.Sigmoid
TANH = mybir.ActivationFunctionType.Tanh

_NC_CACHE = {}
WARM = True
C_PSUM = True
DEBUG = False
BRIDGE = True


def build_nc():
    if 'nc' in _NC_CACHE:
        return _NC_CACHE['nc']
    nc = bacc.Bacc("TRN2", target_bir_lowering=False, debug=False)
    emb = nc.dram_tensor("emb", [V, EP], BF16, kind="ExternalInput")
    idx = nc.dram_tensor("idx", [128, TOK // 128], I32, kind="ExternalInput")
    # lhsT layouts, gate-major columns in torch order i,f,g,o
    wih = nc.dram_tensor("wih", [KE, 4 * H], BF16, kind="ExternalInput")
    whh = nc.dram_tensor("whh", [H, 4 * H], BF16, kind="ExternalInput")
    wlin = nc.dram_tensor("wlin", [H, NCLS], BF16, kind="ExternalInput")
    blin = nc.dram_tensor("blin", [NCLS, 1], F32, kind="ExternalInput")
    out = nc.dram_tensor("out", [NCLS, BC], F32, kind="ExternalOutput")
    if DEBUG:
        dbg_if = nc.dram_tensor("dbg_if", [128, 2, BC], F32, kind="ExternalOutput")
        dbg_go = nc.dram_tensor("dbg_go", [128, 2, BC], F32, kind="ExternalOutput")
        dbg_sif = nc.dram_tensor("dbg_sif", [128, 2, BC], F32, kind="ExternalOutput")
        dbg_g = nc.dram_tensor("dbg_g", [128, BC], F32, kind="ExternalOutput")
        dbg_h = nc.dram_tensor("dbg_h", [128, BC], F32, kind="ExternalOutput")
        dbg_xe = nc.dram_tensor("dbg_xe", [128, 128], F32, kind="ExternalOutput")

    with tile.TileContext(nc) as tc, ExitStack() as ctx:
        singles = ctx.enter_context(tc.tile_pool(name="singles", bufs=1))
        gathp = ctx.enter_context(tc.tile_pool(name="gath", bufs=2))
        xep = ctx.enter_context(tc.tile_pool(name="xeT", bufs=2))
        psump = ctx.enter_context(tc.tile_pool(name="ps", bufs=3, space="PSUM"))
        psumc = ctx.enter_context(tc.tile_pool(name="psc", bufs=2, space="PSUM"))
        gatep = ctx.enter_context(tc.tile_pool(name="gates", bufs=3))
        tmpp = ctx.enter_context(tc.tile_pool(name="tmp", bufs=4))
        statep = ctx.enter_context(tc.tile_pool(name="state", bufs=3))

        idx_sb = singles.tile([128, TOK // 128], I32)
        nc.sync.dma_start(out=idx_sb[:], in_=idx[:, :])
        wih_sb = singles.tile([KE, 4 * H], BF16)
        nc.sync.dma_start(out=wih_sb[:], in_=wih[:, :])
        whh_sb = singles.tile([H, 4 * H], BF16)
        nc.sync.dma_start(out=whh_sb[:], in_=whh[:, :])
        wlin_sb = singles.tile([H, NCLS], BF16)
        nc.sync.dma_start(out=wlin_sb[:], in_=wlin[:, :])
        blin_sb = singles.tile([NCLS, 1], F32)
        nc.sync.dma_start(out=blin_sb[:], in_=blin[:, :])

        dconst = singles.tile([128, 288], BF16)
        nc.vector.memset(dconst[:], 0.25)

        h_prev = statep.tile([H, BC], BF16, tag="h")
        nc.vector.memset(h_prev[:], 0.0)
        cpool = psumc if C_PSUM else statep
        c_prev = cpool.tile([H, BC], F32, tag="c")
        nc.vector.memset(c_prev[:], 0.0)

        xeT_tiles = [None] * NBLK

        def emit_block_gather(b):
            gath = gathp.tile([128, TPB, EP], BF16, tag="gath")
            for j in range(TPB):
                nc.gpsimd.indirect_dma_start(
                    out=gath[:, j, :],
                    out_offset=None,
                    in_=emb[:, :],
                    in_offset=bass.IndirectOffsetOnAxis(
                        ap=idx_sb[:, b * TPB + j:b * TPB + j + 1], axis=0),
                )
            xeT = xep.tile([128, BLK], BF16, tag="xeT")
            for j in range(TPB):
                nc.sync.dma_start(
                    out=xeT[:, j * 128:(j + 1) * 128],
                    in_=gath[:, j, :],
                    transpose=True,
                )
            xeT_tiles[b] = xeT

        def alloc_window_tiles():
            # step-major packing, flat 2D tiles so every AP is a plain range:
            # ps_if cols [j*128, j*128+64) = i, [j*128+64, (j+1)*128) = f;
            # ps_go cols [j*64, (j+1)*64) = g, [256+j*64, ...) = o
            ps_if = psump.tile([128, WSTEPS * 2 * BC], F32, tag="ps_if")
            ps_go = psump.tile([128, 2 * WSTEPS * BC], F32, tag="ps_go")
            return ps_if, ps_go

        def gx_dest(tiles, k, j):
            ps_if, ps_go = tiles
            return (ps_if[:, j * 128:j * 128 + 64],
                    ps_if[:, j * 128 + 64:(j + 1) * 128],
                    ps_go[:, j * BC:(j + 1) * BC],
                    ps_go[:, 256 + j * BC:256 + (j + 1) * BC])[k]

        def emit_gx(tiles, w, k):
            # one gate's input projections (+bias) for window w, one matmul
            # per step so every PSUM write is a contiguous 2D range
            b = w // WPB
            wcol = (w % WPB) * WSTEPS * BC
            for j in range(WSTEPS):
                # start=True exactly once per PSUM bank per window (k==0 is
                # the first matmul into ps_if, k==2 the first into ps_go);
                # start resets the whole bank lazily, so later first-touch
                # writes overwrite and the rec matmuls accumulate.
                nc.tensor.matmul(
                    out=gx_dest(tiles, k, j),
                    lhsT=wih_sb[:, k * H:(k + 1) * H],
                    rhs=xeT_tiles[b][0:KE, wcol + j * BC:wcol + (j + 1) * BC],
                    start=(k in (0, 2) and j == 0), stop=False,
                    skip_group_check=True,
                )

        emit_block_gather(0)
        if NBLK > 1:
            emit_block_gather(1)
        cur = alloc_window_tiles()
        for k in range(4):
            emit_gx(cur, 0, k)

        for w in range(NWIN):
            b = w // WPB
            if w % WPB == WPB - 1 and b + 2 < NBLK and xeT_tiles[b + 2] is None:
                emit_block_gather(b + 2)
            nxt = alloc_window_tiles() if w + 1 < NWIN else None
            ps_if, ps_go = cur
            for j in range(WSTEPS):
                cs = slice(j * BC, (j + 1) * BC)
                # recurrence matmuls: i, f first so sigmoid(i,f) starts early
                nc.tensor.matmul(
                    out=gx_dest(cur, 0, j), lhsT=whh_sb[:, 0:H],
                    rhs=h_prev[:, :], start=False, stop=True,
                    skip_group_check=True)
                nc.tensor.matmul(
                    out=gx_dest(cur, 1, j), lhsT=whh_sb[:, H:2 * H],
                    rhs=h_prev[:, :], start=False, stop=True,
                    skip_group_check=True)
                nc.tensor.matmul(
                    out=gx_dest(cur, 2, j), lhsT=whh_sb[:, 2 * H:3 * H],
                    rhs=h_prev[:, :], start=False, stop=True,
                    skip_group_check=True)
                nc.tensor.matmul(
                    out=gx_dest(cur, 3, j), lhsT=whh_sb[:, 3 * H:4 * H],
                    rhs=h_prev[:, :], start=False, stop=True,
                    skip_group_check=True)
                # hide next window's bulk gx in the tail of this step
                if nxt is not None:
                    emit_gx(nxt, w + 1, j)

                sif = gatep.tile([H, 2 * BC], BF16, tag="sif")
                nc.scalar.activation(
                    out=sif[:, :], in_=ps_if[:, j * 128:(j + 1) * 128],
                    func=SIG)
                gt = gatep.tile([H, BC], BF16, tag="gt")
                nc.scalar.activation(out=gt[:, :], in_=ps_go[:, j * BC:(j + 1) * BC],
                                     func=TANH)
                so = gatep.tile([H, BC], BF16, tag="so")
                nc.scalar.activation(
                    out=so[:, :], in_=ps_go[:, 256 + j * BC:256 + (j + 1) * BC],
                    func=SIG)
                if BRIDGE:
                    # keeps the Activation engine busy across the cell-update
                    # so tanh(c) dispatches back-to-back (no engine wake-up)
                    dscr = tmpp.tile([128, 288], BF16, tag="dscr")
                    nc.scalar.activation(out=dscr[:, :], in_=dconst[:, :],
                                         func=TANH)

                t2 = tmpp.tile([H, BC], F32, tag="t2")
                nc.vector.tensor_mul(t2[:], sif[:, BC:2 * BC], c_prev[:, :])
                t1 = tmpp.tile([H, BC], F32, tag="t1")
                nc.vector.tensor_mul(t1[:], sif[:, 0:BC], gt[:, :])
                c_new = cpool.tile([H, BC], F32, tag="c")
                nc.vector.tensor_add(c_new[:], t1[:], t2[:])
                tc_t = tmpp.tile([H, BC], BF16, tag="tc")
                nc.scalar.activation(out=tc_t[:], in_=c_new[:, :], func=TANH)
                if WARM:
                    # small matmul on tanh(c): keeps PE activity (and device
                    # clocks) up through the serial tail; output lands on the
                    # already-consumed o column, start=False leaves the
                    # bank's pending-zero state untouched
                    nc.tensor.matmul(
                        out=ps_go[0:1, 256 + j * BC:256 + (j + 1) * BC],
                        lhsT=wlin_sb[:, 0:1],
                        rhs=tc_t[:, :], start=False, stop=True,
                        skip_group_check=True)
                h_new = statep.tile([H, BC], BF16, tag="h")
                nc.vector.tensor_mul(h_new[:], so[:, :], tc_t[:])
                if DEBUG and w == 0 and j == 0:
                    dbg1 = tmpp.tile([128, 2, BC], F32, tag="dbg1")
                    nc.scalar.activation(out=dbg1[:], in_=ps_if[:, 0:128],
                                         func=mybir.ActivationFunctionType.Identity)
                    nc.sync.dma_start(out=dbg_if[:], in_=dbg1[:])
                    dbg2 = tmpp.tile([128, 2, BC], F32, tag="dbg2")
                    nc.scalar.activation(out=dbg2[:, 0, :], in_=ps_go[:, 0:BC],
                                         func=mybir.ActivationFunctionType.Identity)
                    nc.scalar.activation(out=dbg2[:, 1, :], in_=ps_go[:, 256:256 + BC],
                                         func=mybir.ActivationFunctionType.Identity)
                    nc.sync.dma_start(out=dbg_go[:], in_=dbg2[:])
                    dbg3 = tmpp.tile([128, 2, BC], F32, tag="dbg3")
                    nc.vector.tensor_copy(dbg3[:], sif[:, :])
                    nc.sync.dma_start(out=dbg_sif[:], in_=dbg3[:])
                    dbg4 = tmpp.tile([128, BC], F32, tag="dbg4")
                    nc.vector.tensor_copy(dbg4[:], gt[:, :])
                    nc.sync.dma_start(out=dbg_g[:], in_=dbg4[:])
                    dbg5 = tmpp.tile([128, BC], F32, tag="dbg5")
                    nc.vector.tensor_copy(dbg5[:], h_new[:, :])
                    nc.sync.dma_start(out=dbg_h[:], in_=dbg5[:])
                    dbg6 = tmpp.tile([128, 128], F32, tag="dbg6")
                    nc.vector.tensor_copy(dbg6[:], xeT_tiles[0][:, 0:128])
                    nc.sync.dma_start(out=dbg_xe[:], in_=dbg6[:])
                h_prev, c_prev = h_new, c_new
            cur = nxt

        ps_f = psumc.tile([H, BC], F32, tag="c")
        nc.tensor.matmul(out=ps_f[0:NCLS, 0:BC], lhsT=wlin_sb[:, :],
                         rhs=h_prev[:, :], start=True, stop=True,
                         skip_group_check=True)
        out_sb = tmpp.tile([NCLS, BC], F32, tag="outsb")
        nc.scalar.activation(
            out=out_sb[:, :], in_=ps_f[0:NCLS, 0:BC],
            func=mybir.ActivationFunctionType.Identity,
            bias=blin_sb[:, :],
        )
        nc.sync.dma_start(out=out[:, :], in_=out_sb[:, :])

    nc.compile()
    _NC_CACHE['nc'] = nc
    return nc


def prep_inputs(x, emb_table, w_ih, w_hh, b_ih, b_hh, w_lin, b_lin):
    """Host-side prep: shard batch, lhsT weight layouts (torch gate order
    i,f,g,o), bias folded into an extra w_ih row against the embedding's
    constant-1.0 column."""
    import ml_dtypes
    x = np.asarray(x)
    emb_f32 = np.asarray(emb_table, dtype=np.float32)
    emb_pad = np.zeros((V, EP), dtype=ml_dtypes.bfloat16)
    emb_pad[:, :E] = emb_f32.astype(ml_dtypes.bfloat16)
    emb_pad[:, E] = 1.0
    w_ih = np.asarray(w_ih, dtype=np.float32)
    w_hh = np.asarray(w_hh, dtype=np.float32)
    bias = np.asarray(b_ih, dtype=np.float32) + np.asarray(b_hh, dtype=np.float32)
    w_lin = np.asarray(w_lin, dtype=np.float32)
    b_lin = np.asarray(b_lin, dtype=np.float32)

    wih_lhsT = np.zeros((KE, 4 * H), dtype=ml_dtypes.bfloat16)
    wih_lhsT[:E] = w_ih.T.astype(ml_dtypes.bfloat16)
    wih_lhsT[E] = bias.astype(ml_dtypes.bfloat16)
    whh_lhsT = np.ascontiguousarray(w_hh.T).astype(ml_dtypes.bfloat16)
    wlin_lhsT = np.ascontiguousarray(w_lin.T).astype(ml_dtypes.bfloat16)
    blin = np.ascontiguousarray(b_lin.reshape(NCLS, 1))

    in_maps = []
    for c in range(NCORES):
        xc = x[c * BC:(c + 1) * BC]                         # [BC, T]
        toks = np.ascontiguousarray(xc.T).reshape(-1)       # t-major: k = t*BC + b
        idx_host = np.ascontiguousarray(
            toks.astype(np.int32).reshape(TOK // 128, 128).T)  # [128, TOK/128]
        in_maps.append({
            "emb": emb_pad,
            "idx": idx_host,
            "wih": wih_lhsT,
            "whh": whh_lhsT,
            "wlin": wlin_lhsT,
            "blin": blin,
        })
    return in_maps


def run(inputs, trace=False):
    nc = build_nc()
    in_maps = prep_inputs(**inputs)
    res = run_bass_kernel_spmd(nc, in_maps, core_ids=list(range(NCORES)),
                               trace=trace)
    outs = [r["out"] for r in res.results]                  # each [NCLS, BC]
    full = np.concatenate([o.T for o in outs], axis=0)      # [B, NCLS]
    return full.astype(np.float32), res


def kernel(**inputs):
    out, _ = run(inputs, trace=False)
    return out
